# revision 10
# baseline (speedup 1.0000x reference)
"""MultiHeadInfiniAttention Trainium2 kernel.

Reference computation (B=4, S=8192, D=1024, H=8, dk=dv=128, SEG=512):
  q,k,v = x @ w? + b?            -> per (seg, batch, head): [512, 128]
  scan over 16 segments with per-(b,h) memory state:
    sk = elu(k)+1; mem += sk^T v; z += SEG * sum_l(sk)
    sq = elu(q)+1; a_mem = (sq mem)/(sq z + 1e-5)
    attn = softmax(q k^T / sqrt(dk)); a_dot = attn v
    out = sig(beta) * mean_h(a_mem) + (1-sig(beta)) * mean_h(a_dot)

Sharding: 8 cores = 4 batches x 2 head-groups (4 heads each). Each core
computes a partial head-sum [S, 128]; host adds the two halves per batch.
The blend coefficients (from beta) are folded in on-chip: SEG/cb scales the
z accumulation column (so the retrieval num/den ratio carries cb), 1/cd
(cd = (1-sig)/H) is planted in the softmax denominator column, and the v
projection bias collapses analytically to a constant output row
(cb/SEG + cd) * sum_h bv_h that seeds out_acc.

Layouts on chip (per segment t == seq tile of 512):
  xT tile   [128d, 8 dt, 512s] f32r, single 3D DMA straight from DRAM
            (f32r is bit-identical to f32; DMA-direct works on HW)
  qT/kT     [128 dk, 512 l] f32r, transposed projection (w stationary);
            bias is a per-partition scalar in this orientation; k is
            projected first so the sk_nat transposes start early
  sqT/skT   [128 dk, 512 l] bf16   (elu+1 = relu(x) + min(exp(x), 1))
  sk_nat    [128 l, 4 lt, 128 dk]: ONE partition-tiled dma transpose of
            skT per head (16 -> 4 transposes/segment; keeps the sync DMA
            ring unsaturated)
  v         natural-orientation projection (xT block stationary, N=512
            covers all 4 heads), scattered into v_ext blocks; bias folded
            into the out_acc initializer (see module blurb)
  v_ext     persistent, 16 blocks x 160 cols: 0:128 = v, col 128 = SEG/cb,
            col 129 = 1/cd (constant cols written once in the preamble)
  scores^T  psum [128 m, 512 l] = kT_block.T @ qT   (f32r, N=512 full rate)
  expT      [128 m, 512 l] bf16 = Exp(scores/sqrt(dk))
  mem update psum [128 dk, 129] = sum_lt sk_nat.T @ v_ext_lt[:, :129]
  mem_bf    [128 dk, 129] bf16 persistent per head, accumulated directly
            in bf16 (cols 0:128 = mem, col 128 = z * SEG/cb); no f32
            staging or per-head repack
  a_dot     psum [128 l, 130] = sum_mt expT_mt.T @ v_ext_mt[:, :130]
            (denominator lands in col 129; col 128 is junk)
  retrieval psum [128 l, 129] = sqT_block.T @ mem_bf (issued after the
            adot group so the mem_bf DVE accumulate is hidden)
  out_acc   [128 l, 128] x4 f32, seeded with the folded v-bias row and
            accumulated over heads via out += numer * recip(denom) DVE
            ops; the reference's +1e-5 on the z denominator is dropped
            (denominator >= ~1e5)

Hardware notes learned the hard way:
  - matmul start=True clears the has_written state of the WHOLE psum bank,
    so a bank must host exactly one accumulation group at a time.
  - dma_start_transpose destinations must be 64B-aligned.
  - scalar_tensor_tensor has no divide in the ISA op set.
  - DMA transposes and bulk loads on the Activation HWDGE rail stall the
    ACT compute stream badly; keep them on the sync rail.
  - SWDGE (gpsimd) DMA is slow (~75 GB/s) -- fine for preamble weight
    streaming and the out store, not for anything latency-critical.
  - The PE p-state ramps with sustained activity: dribbling matmuls in
    behind a slow weight stream keeps the clock low; front-load big DMAs.
"""

import sys

for _p in ("/opt/trn_rl_repo",):
    if _p not in sys.path:
        sys.path.insert(0, _p)

import numpy as np

import concourse.bass as bass
import concourse.tile as tile
import concourse.mybir as mybir
from concourse import bacc
from concourse.bass_utils import run_bass_kernel_spmd

F32 = mybir.dt.float32
F32R = mybir.dt.float32r
BF16 = mybir.dt.bfloat16
AF = mybir.ActivationFunctionType
ALU = mybir.AluOpType

B, S, D = 4, 8192, 1024
H_TOT, DK, DV, SEG = 8, 128, 128, 512
NSEG = S // SEG          # 16
HPC = 4                  # heads per core
NDT = D // 128           # 8 d-tiles
NCORES = 8
SCALE = 1.0 / float(np.sqrt(DK))

_CACHE = {}
DEBUG_TAPS = False
REPS = 1  # repeat the whole segment loop (for HW timing via deltas)
ABL = frozenset()  # ablation flags (timing experiments only; empty = real)


def _build():
    nc = bacc.Bacc("TRN2", target_bir_lowering=False, debug=False,
                   num_devices=NCORES)

    xT = nc.dram_tensor("xT", [D, S], F32R, kind="ExternalInput")
    wq = nc.dram_tensor("wq", [D, HPC * DK], F32R, kind="ExternalInput")
    wk = nc.dram_tensor("wk", [D, HPC * DK], F32R, kind="ExternalInput")
    wv = nc.dram_tensor("wv", [D, HPC * DV], F32R, kind="ExternalInput")
    bq = nc.dram_tensor("bq", [HPC, DK], F32, kind="ExternalInput")
    bk = nc.dram_tensor("bk", [HPC, DK], F32, kind="ExternalInput")
    # bias_out: [128, 128] broadcast rows of (cb/SEG + cd) * sum_h bv_h —
    # the analytically folded v-bias contribution to the output (softmax
    # rows sum to 1; the memory path's bias term collapses to bv/SEG).
    bias_out = nc.dram_tensor("bias_out", [128, DV], F32, kind="ExternalInput")
    # coef columns: 0 = cb (= sig/H), 1 = 1/cd (cd = (1-sig)/H),
    # 2 = SEG/cb (z accumulation factor with cb pre-divided so the
    # retrieval numerator/denominator ratio carries the cb scale),
    # broadcast 128x
    coef = nc.dram_tensor("coef", [128, 3], F32, kind="ExternalInput")
    out = nc.dram_tensor("out", [S, DV], F32, kind="ExternalOutput")

    dbg = {}
    if DEBUG_TAPS:
        for nm, shp in (("q", [128, 512]), ("k", [128, 512]),
                        ("sq", [128, 512]), ("skn", [128, 2048]),
                        ("vex", [128, 2048]), ("exp", [128, 2048]),
                        ("mem", [128, 129]), ("rtp", [128, 129]),
                        ("adp", [128, 129]), ("oacc", [128, 512]),
                        ("up", [128, 130])):
            dbg[nm] = nc.dram_tensor(f"dbg_{nm}", shp, BF16,
                                     kind="ExternalOutput")

    with tile.TileContext(nc) as tc:
        with tc.tile_pool(name="const", bufs=1) as cpool, \
             tc.tile_pool(name="work", bufs=2) as wpool, \
             tc.tile_pool(name="small", bufs=8) as spool, \
             tc.tile_pool(name="pp", bufs=3, space="PSUM") as proj_ps, \
             tc.tile_pool(name="sp", bufs=2, space="PSUM") as score_ps, \
             tc.tile_pool(name="sml_ps", bufs=3, space="PSUM") as sml_ps:
            upd_ps = ret_ps = adot_ps = sml_ps

            # ---- preamble: weights stream on the SWDGE rail (parallel to
            # the sync ring, which then reaches segment 0's x load after
            # only the tiny bias/coef/oinit transfers); k first because the
            # projection loop consumes k first. ----
            w_r = {}
            b_sb = {}
            for name, wd, bd in (("k", wk, bk), ("q", wq, bq), ("v", wv, None)):
                wr = cpool.tile([128, NDT * 512], F32R, name=f"wr_{name}")
                for dt in range(NDT):
                    nc.gpsimd.dma_start(wr[:, dt * 512:(dt + 1) * 512],
                                        wd.ap()[dt * 128:(dt + 1) * 128, :])
                w_r[name] = wr
                if bd is None:
                    continue
                bt = cpool.tile([128, HPC], F32, name=f"b_{name}")
                for j in range(HPC):
                    nc.sync.dma_start(
                        bt[:, j:j + 1],
                        bd.ap()[j:j + 1, :].rearrange("a p -> p a"))
                b_sb[name] = bt

            coef_sb = cpool.tile([128, 3], F32, name="coef_sb")
            nc.sync.dma_start(coef_sb[:], coef.ap())

            # cz: col 0 = SEG/cb (z accumulation factor), col 1 = 1/cd
            # (softmax denominator scale); constant over partitions.
            cz = cpool.tile([128, 2], BF16, name="cz")
            nc.vector.tensor_copy(cz[:, 0:1], coef_sb[:, 2:3])
            nc.vector.tensor_copy(cz[:, 1:2], coef_sb[:, 1:2])

            # out_acc initializer: the folded v-bias row replicated into all
            # four l-blocks (see bias_out above).
            oinit = cpool.tile([128, 4 * DV], F32, name="oinit")
            for lb in range(4):
                nc.sync.dma_start(oinit[:, lb * DV:(lb + 1) * DV],
                                  bias_out.ap())

            # memory state per head, bf16: cols 0:128 = mem, col 128 = z
            # (z pre-scaled by 1/cb via the SEG/cb constant column).
            mem_bf = cpool.tile([128, HPC * 129], BF16, name="mem_bf")

            # two persistent v_ext buffers (alternate per segment); constant
            # cols written once here. Blocks of 160 cols: 0:128 = v,
            # col 128 = SEG/cb, col 129 = 1/cd, 130:160 unused.
            v_ext_ab = []
            for i in range(2):
                ve = cpool.tile([128, HPC * 4 * 160], BF16, name=f"vext{i}")
                for blk in range(HPC * 4):
                    nc.gpsimd.tensor_copy(
                        ve[:, blk * 160 + 128:blk * 160 + 130], cz[:])
                v_ext_ab.append(ve)

            # ---------- software-pipelined segment loop helpers ----------
            def alloc_seg(rep, t):
                u = rep * NSEG + t  # unique id for tile names
                st = {"t": t}
                st["xt"] = wpool.tile([128, NDT * 512], F32R,
                                      name=f"xtr{u}", tag="xtr")
                nc.sync.dma_start(
                    st["xt"][:].rearrange("p (dt s) -> p dt s", s=512),
                    xT.ap()[:, t * 512:(t + 1) * 512]
                      .rearrange("(dt p) s -> p dt s", p=128))
                st["qT"] = wpool.tile([128, HPC * 512], F32R,
                                      name=f"qT{u}", tag="qT")
                st["kT"] = wpool.tile([128, HPC * 512], F32R,
                                      name=f"kT{u}", tag="kT")
                st["sqT"] = wpool.tile([128, HPC * 512], BF16,
                                       name=f"sqT{u}", tag="sqT")
                st["skT"] = wpool.tile([128, HPC * 512], BF16,
                                       name=f"skT{u}", tag="skT")
                st["skn"] = wpool.tile([128, HPC * 512], BF16,
                                       name=f"skn{u}", tag="skn")
                st["v_ext"] = v_ext_ab[t % 2]
                st["oacc"] = wpool.tile([128, 4 * 128], F32,
                                        name=f"oac{u}", tag="oacc")
                nc.gpsimd.tensor_copy(st["oacc"][:], oinit[:])
                st["u"] = u
                return st

            def emit_kq_group(st, name, hb):
                if "no_kq" in ABL:
                    return
                u = st["u"]
                wr = w_r[name]
                pp = proj_ps.tile([128, 512], F32,
                                  name=f"pp{u}_{name}{hb}", tag="proj")
                for dt in range(NDT):
                    nc.tensor.matmul(
                        pp[:],
                        wr[:, dt * 512 + hb * 128:dt * 512 + (hb + 1) * 128],
                        st["xt"][:, dt * 512:(dt + 1) * 512],
                        start=(dt == 0), stop=(dt == NDT - 1))
                bias = b_sb[name][:, hb:hb + 1]
                sl = slice(hb * 512, (hb + 1) * 512)
                raw = st["qT"] if name == "q" else st["kT"]
                s_out = st["sqT"] if name == "q" else st["skT"]
                # raw projection with bias (f32r, scores operand)
                if "no_raw" not in ABL:
                    nc.scalar.activation(raw[:, sl], pp[:], AF.Identity,
                                         bias=bias)
                # elu(x)+1 = relu(x) + min(exp(x), 1)
                if "no_feat" not in ABL:
                    e_t = spool.tile([128, 512], BF16,
                                     name=f"e{u}_{name}{hb}", tag="e", bufs=3)
                    nc.scalar.activation(e_t[:], pp[:], AF.Exp, bias=bias)
                    r_t = spool.tile([128, 512], BF16,
                                     name=f"r{u}_{name}{hb}", tag="r", bufs=3)
                    nc.vector.tensor_scalar(r_t[:], pp[:], bias, 0.0,
                                            op0=ALU.add, op1=ALU.max)
                    # s_out = min(e, 1) + r in one fused op
                    nc.vector.scalar_tensor_tensor(
                        s_out[:, sl], e_t[:], 1.0, r_t[:],
                        op0=ALU.min, op1=ALU.add)
                if name == "k" and "no_tp" not in ABL:
                    # one block-transpose per head: [128 dk, 512 l]
                    # -> [128 l, 4 lt, 128 dk] partition-tiled
                    nc.sync.dma_start_transpose(
                        st["skn"][:, hb * 512:(hb + 1) * 512]
                        .rearrange("p (lt c) -> p lt c", c=128),
                        st["skT"][:, hb * 512:(hb + 1) * 512])

            def emit_v_group(st, lt):
                if "no_v" in ABL:
                    return
                u = st["u"]
                wr = w_r["v"]
                pp = proj_ps.tile([128, 512], F32,
                                  name=f"pp{u}_v{lt}", tag="proj")
                for dt in range(NDT):
                    nc.tensor.matmul(
                        pp[:],
                        st["xt"][:, dt * 512 + lt * 128:dt * 512 + (lt + 1) * 128],
                        wr[:, dt * 512:(dt + 1) * 512],
                        start=(dt == 0), stop=(dt == NDT - 1))
                # scatter the 4 head blocks into v_ext (block id lt*4+h)
                if "no_vscat" not in ABL:
                    nc.scalar.activation(
                        st["v_ext"][:, lt * 640:(lt + 1) * 640]
                        .rearrange("p (h c) -> p h c", c=160)[:, :, 0:128],
                        pp[:].rearrange("p (h c) -> p h c", c=128),
                        AF.Copy)

            def emit_attn_head(st, h):
                u = st["u"]
                kT, qT = st["kT"], st["qT"]
                sqT, sk_nat = st["sqT"], st["skn"]
                v_ext, out_acc = st["v_ext"], st["oacc"]
                hsl = slice(h * 512, (h + 1) * 512)
                # scores^T -> exp
                exp_sb = wpool.tile([128, 4 * 512], BF16,
                                    name=f"ex{u}_{h}", tag="exp")
                if "no_scores" not in ABL:
                    for mb in range(4):
                        sps = score_ps.tile([128, 512], F32,
                                            name=f"sc{u}_{h}{mb}", tag="score")
                        nc.tensor.matmul(
                            sps[:],
                            kT[:, h * 512 + mb * 128:h * 512 + (mb + 1) * 128],
                            qT[:, hsl], start=True, stop=True)
                        if "no_exp" not in ABL:
                            nc.scalar.activation(
                                exp_sb[:, mb * 512:(mb + 1) * 512],
                                sps[:], AF.Exp, scale=SCALE)
                # memory update; the bf16 accumulate hides behind the
                # first adot group below
                msl = slice(h * 129, (h + 1) * 129)
                if "no_update" not in ABL:
                    up = upd_ps.tile([128, 129], F32, name=f"up{u}_{h}",
                                     tag="sml")
                    for lt in range(4):
                        blk = slice((h * 4 + lt) * 128, (h * 4 + lt + 1) * 128)
                        vbase = (lt * 4 + h) * 160
                        nc.tensor.matmul(
                            up[:], sk_nat[:, blk],
                            v_ext[:, vbase:vbase + 129],
                            start=(lt == 0), stop=(lt == 3))
                    nc.vector.tensor_add(mem_bf[:, msl], mem_bf[:, msl],
                                         up[:])
                for lb in range(4):
                    lsl = slice(h * 512 + lb * 128, h * 512 + (lb + 1) * 128)
                    adp = None
                    if "no_adot" not in ABL:
                        adp = adot_ps.tile([128, 130], F32,
                                           name=f"ad{u}_{h}{lb}", tag="sml")
                        for mt in range(4):
                            esl = slice(mt * 512 + lb * 128,
                                        mt * 512 + (lb + 1) * 128)
                            vbase = (mt * 4 + h) * 160
                            nc.tensor.matmul(
                                adp[:], exp_sb[:, esl],
                                v_ext[:, vbase:vbase + 130],
                                start=(mt == 0), stop=(mt == 3))
                    rps = None
                    if "no_ret" not in ABL:
                        rps = ret_ps.tile([128, 129], F32,
                                          name=f"rt{u}_{h}{lb}", tag="sml")
                        nc.tensor.matmul(rps[:], sqT[:, lsl], mem_bf[:, msl],
                                         start=True, stop=True)
                    if "no_epi" in ABL:
                        continue
                    # epilogue: out += cb*numer/denz + numer2/dend
                    # (reference's +1e-5 is negligible: denz >= ~1e5)
                    osl = out_acc[:, lb * 128:(lb + 1) * 128]
                    if rps is not None:
                        rz = spool.tile([128, 1], F32, name=f"rz{u}_{h}{lb}",
                                        tag="rz", bufs=8)
                        nc.vector.reciprocal(rz[:], rps[:, 128:129])
                        nc.vector.scalar_tensor_tensor(
                            osl, rps[:, 0:128], rz[:], osl,
                            op0=ALU.mult, op1=ALU.add)
                    if adp is not None:
                        rd = spool.tile([128, 1], F32, name=f"rd{u}_{h}{lb}",
                                        tag="rd", bufs=8)
                        nc.vector.reciprocal(rd[:], adp[:, 129:130])
                        nc.vector.scalar_tensor_tensor(
                            osl, adp[:, 0:128], rd[:], osl,
                            op0=ALU.mult, op1=ALU.add)

            def emit_out_store(st):
                t = st["t"]
                nc.gpsimd.dma_start(
                    out.ap()[t * 512:(t + 1) * 512, :]
                       .rearrange("(lt p) v -> p lt v", p=128),
                    st["oacc"][:].rearrange("p (lt v) -> p lt v", v=128))

            # ---------- pipelined emission: proj(t) interleaved with
            # attention(t-1) so PE always has ready work while ACT runs
            # the exps of the previous segment's attention ----------
            for rep in range(REPS):
                nc.vector.memset(mem_bf[:], 0.0)
                prev = None
                for t in range(NSEG):
                    st = alloc_seg(rep, t)
                    emit_kq_group(st, "k", 0)
                    emit_kq_group(st, "k", 1)
                    if prev is not None:
                        emit_attn_head(prev, 0)
                    emit_kq_group(st, "k", 2)
                    emit_kq_group(st, "k", 3)
                    if prev is not None:
                        emit_attn_head(prev, 1)
                    emit_kq_group(st, "q", 0)
                    emit_kq_group(st, "q", 1)
                    if prev is not None:
                        emit_attn_head(prev, 2)
                    emit_kq_group(st, "q", 2)
                    emit_kq_group(st, "q", 3)
                    if prev is not None:
                        emit_attn_head(prev, 3)
                    emit_v_group(st, 0)
                    emit_v_group(st, 1)
                    if prev is not None:
                        emit_out_store(prev)
                    emit_v_group(st, 2)
                    emit_v_group(st, 3)
                    prev = st
                # drain: attention of the last segment
                for h in range(HPC):
                    emit_attn_head(prev, h)
                emit_out_store(prev)

    nc.compile()
    return nc


def _get_compiled():
    if "nc" not in _CACHE:
        _CACHE["nc"] = _build()
    return _CACHE["nc"]


def make_in_maps(x, wq, bq, wk, bk, wv, bv, beta):
    bsig = float(1.0 / (1.0 + np.exp(-np.float64(beta[0]))))
    cb = bsig / H_TOT
    cd = (1.0 - bsig) / H_TOT
    coef = np.empty((128, 3), np.float32)
    coef[:, 0] = cb
    coef[:, 1] = (1.0 / cd) if cd != 0.0 else np.inf
    coef[:, 2] = SEG / cb

    xT_by_b = [np.ascontiguousarray(x[b].T).astype(np.float32, copy=False)
               for b in range(B)]
    in_maps = []
    for c in range(NCORES):
        b, hg = c // 2, c % 2
        sl = slice(hg * HPC * DK, (hg + 1) * HPC * DK)
        bv_core = np.ascontiguousarray(bv[sl]).reshape(HPC, DV)
        brow = (cb / SEG + cd) * bv_core.sum(axis=0)
        bias_out = np.broadcast_to(brow.astype(np.float32),
                                   (128, DV)).copy()
        in_maps.append({
            "xT": xT_by_b[b],
            "wq": np.ascontiguousarray(wq[:, sl]),
            "wk": np.ascontiguousarray(wk[:, sl]),
            "wv": np.ascontiguousarray(wv[:, sl]),
            "bq": np.ascontiguousarray(bq[sl]).reshape(HPC, DK),
            "bk": np.ascontiguousarray(bk[sl]).reshape(HPC, DK),
            "bias_out": bias_out,
            "coef": coef,
        })
    return in_maps


def kernel(x, wq, bq, wk, bk, wv, bv, beta):
    nc = _get_compiled()
    in_maps = make_in_maps(x, wq, bq, wk, bk, wv, bv, beta)
    res = run_bass_kernel_spmd(nc, in_maps, core_ids=list(range(NCORES)))
    out = np.empty((B, S, DV), np.float32)
    for b in range(B):
        out[b] = res.results[2 * b]["out"] + res.results[2 * b + 1]["out"]
    return out


if __name__ == "__main__":
    rng = np.random.default_rng(0)
    x = rng.normal(size=(B, S, D)).astype(np.float32)
    sc = 1.0 / np.sqrt(D)
    wq_ = (rng.normal(size=(D, 1024)) * sc).astype(np.float32)
    wk_ = (rng.normal(size=(D, 1024)) * sc).astype(np.float32)
    wv_ = (rng.normal(size=(D, 1024)) * sc).astype(np.float32)
    bq_ = (rng.normal(size=(1024,)) * 0.01).astype(np.float32)
    bk_ = (rng.normal(size=(1024,)) * 0.01).astype(np.float32)
    bv_ = (rng.normal(size=(1024,)) * 0.01).astype(np.float32)
    beta_ = np.zeros((1,), np.float32)
    o = kernel(x, wq_, bq_, wk_, bk_, wv_, bv_, beta_)
    print("out", o.shape, o.dtype, float(np.abs(o).max()))



# revision 24
# speedup vs baseline: 1.0196x; 1.0196x over previous
"""MultiHeadInfiniAttention Trainium2 kernel.

Reference computation (B=4, S=8192, D=1024, H=8, dk=dv=128, SEG=512):
  q,k,v = x @ w? + b?            -> per (seg, batch, head): [512, 128]
  scan over 16 segments with per-(b,h) memory state:
    sk = elu(k)+1; mem += sk^T v; z += SEG * sum_l(sk)
    sq = elu(q)+1; a_mem = (sq mem)/(sq z + 1e-5)
    attn = softmax(q k^T / sqrt(dk)); a_dot = attn v
    out = sig(beta) * mean_h(a_mem) + (1-sig(beta)) * mean_h(a_dot)

Sharding: 8 cores = 4 batches x 2 head-groups (4 heads each). Each core
computes a partial head-sum [S, 128]; host adds the two halves per batch.
The blend coefficients (from beta) are folded in on-chip: SEG/cb scales the
z accumulation column (so the retrieval num/den ratio carries cb), 1/cd
(cd = (1-sig)/H) is planted in the softmax denominator column, and the v
projection bias collapses analytically to a constant output row
(cb/SEG + cd) * sum_h bv_h that seeds out_acc.

Layouts on chip (per segment t == seq tile of 512):
  xT tile   [128d, 8 dt, 512s] f32r, single 3D DMA straight from DRAM
            (f32r is bit-identical to f32; DMA-direct works on HW)
  qT/kT     [128 dk, 512 l] f32r, transposed projection (w stationary);
            bias is a per-partition scalar in this orientation; k is
            projected first so the sk_nat transposes start early
  sqT/skT   [128 dk, 512 l] bf16   (elu+1 = relu(x) + min(exp(x), 1))
  sk_nat    [128 l, 4 lt, 128 dk]: ONE partition-tiled dma transpose of
            skT per head (16 -> 4 transposes/segment; keeps the sync DMA
            ring unsaturated)
  v         natural-orientation projection (xT block stationary, N=512
            covers all 4 heads), scattered into v_ext blocks; bias folded
            into the out_acc initializer (see module blurb)
  v_ext     persistent, 16 blocks x 160 cols: 0:128 = v, col 128 = SEG/cb,
            col 129 = 1/cd (constant cols written once in the preamble)
  scores^T  psum [128 m, 512 l] = kT_block.T @ qT   (f32r, N=512 full rate)
  expT      [128 m, 512 l] bf16 = Exp(scores/sqrt(dk))
  mem update psum [128 dk, 129] = sum_lt sk_nat.T @ v_ext_lt[:, :129]
  mem_bf    [128 dk, 129] bf16 persistent per head, accumulated directly
            in bf16 (cols 0:128 = mem, col 128 = z * SEG/cb); no f32
            staging or per-head repack
  a_dot     psum [128 l, 130] = sum_mt expT_mt.T @ v_ext_mt[:, :130]
            (denominator lands in col 129; col 128 is junk)
  retrieval psum [128 l, 129] = sqT_block.T @ mem_bf (issued after the
            adot group so the mem_bf DVE accumulate is hidden)
  out_acc   [128 l, 128] x4 f32, seeded with the folded v-bias row and
            accumulated over heads via out += numer * recip(denom) DVE
            ops; the reference's +1e-5 on the z denominator is dropped
            (denominator >= ~1e5)

Hardware notes learned the hard way:
  - matmul start=True clears the has_written state of the WHOLE psum bank,
    so a bank must host exactly one accumulation group at a time.
  - dma_start_transpose destinations must be 64B-aligned.
  - scalar_tensor_tensor has no divide in the ISA op set.
  - DMA transposes and bulk loads on the Activation HWDGE rail stall the
    ACT compute stream badly; keep them on the sync rail.
  - SWDGE (gpsimd) DMA is slow (~75 GB/s) -- fine for preamble weight
    streaming and the out store, not for anything latency-critical.
  - The PE p-state ramps with sustained activity: dribbling matmuls in
    behind a slow weight stream keeps the clock low; front-load big DMAs.
"""

import sys

for _p in ("/opt/trn_rl_repo",):
    if _p not in sys.path:
        sys.path.insert(0, _p)

import numpy as np

import concourse.bass as bass
import concourse.tile as tile
import concourse.mybir as mybir
from concourse import bacc
from concourse.bass_utils import run_bass_kernel_spmd

F32 = mybir.dt.float32
F32R = mybir.dt.float32r
BF16 = mybir.dt.bfloat16
AF = mybir.ActivationFunctionType
ALU = mybir.AluOpType

B, S, D = 4, 8192, 1024
H_TOT, DK, DV, SEG = 8, 128, 128, 512
NSEG = S // SEG          # 16
HPC = 4                  # heads per core
NDT = D // 128           # 8 d-tiles
NCORES = 8
SCALE = 1.0 / float(np.sqrt(DK))

_CACHE = {}
DEBUG_TAPS = False
REPS = 1  # repeat the whole segment loop (for HW timing via deltas)
ABL = frozenset()  # ablation flags (timing experiments only; empty = real)


def _build():
    nc = bacc.Bacc("TRN2", target_bir_lowering=False, debug=False,
                   num_devices=NCORES)

    xT = nc.dram_tensor("xT", [D, S], F32R, kind="ExternalInput")
    wq = nc.dram_tensor("wq", [D, HPC * DK], F32R, kind="ExternalInput")
    wk = nc.dram_tensor("wk", [D, HPC * DK], F32R, kind="ExternalInput")
    wv = nc.dram_tensor("wv", [D, HPC * DV], F32R, kind="ExternalInput")
    bq = nc.dram_tensor("bq", [HPC, DK], F32, kind="ExternalInput")
    bk = nc.dram_tensor("bk", [HPC, DK], F32, kind="ExternalInput")
    # bias_out: [128, 128] broadcast rows of (cb/SEG + cd) * sum_h bv_h —
    # the analytically folded v-bias contribution to the output (softmax
    # rows sum to 1; the memory path's bias term collapses to bv/SEG).
    bias_out = nc.dram_tensor("bias_out", [128, DV], F32, kind="ExternalInput")
    # coef columns: 0 = cb (= sig/H), 1 = 1/cd (cd = (1-sig)/H),
    # 2 = SEG/cb (z accumulation factor with cb pre-divided so the
    # retrieval numerator/denominator ratio carries the cb scale),
    # broadcast 128x
    coef = nc.dram_tensor("coef", [128, 3], F32, kind="ExternalInput")
    out = nc.dram_tensor("out", [S, DV], F32, kind="ExternalOutput")

    dbg = {}
    if DEBUG_TAPS:
        for nm, shp in (("q", [128, 512]), ("k", [128, 512]),
                        ("sq", [128, 512]), ("skn", [128, 2048]),
                        ("vex", [128, 2048]), ("exp", [128, 2048]),
                        ("mem", [128, 129]), ("rtp", [128, 129]),
                        ("adp", [128, 129]), ("oacc", [128, 512]),
                        ("up", [128, 130])):
            dbg[nm] = nc.dram_tensor(f"dbg_{nm}", shp, BF16,
                                     kind="ExternalOutput")

    with tile.TileContext(nc) as tc:
        with tc.tile_pool(name="const", bufs=1) as cpool, \
             tc.tile_pool(name="work", bufs=2) as wpool, \
             tc.tile_pool(name="small", bufs=8) as spool, \
             tc.tile_pool(name="pp", bufs=2, space="PSUM") as proj_ps, \
             tc.tile_pool(name="sp", bufs=2, space="PSUM") as score_ps, \
             tc.tile_pool(name="sml_ps", bufs=4, space="PSUM") as sml_ps:
            upd_ps = ret_ps = adot_ps = sml_ps

            # ---- preamble: weights stream on the SWDGE rail (parallel to
            # the sync ring, which then reaches segment 0's x load after
            # only the tiny bias/coef/oinit transfers); k first because the
            # projection loop consumes k first. ----
            w_r = {}
            b_sb = {}
            for name, wd, bd in (("k", wk, bk), ("q", wq, bq), ("v", wv, None)):
                wr = cpool.tile([128, NDT * 512], F32R, name=f"wr_{name}")
                for dt in range(NDT):
                    nc.gpsimd.dma_start(wr[:, dt * 512:(dt + 1) * 512],
                                        wd.ap()[dt * 128:(dt + 1) * 128, :])
                w_r[name] = wr
                if bd is None:
                    continue
                bt = cpool.tile([128, HPC], F32, name=f"b_{name}")
                for j in range(HPC):
                    nc.sync.dma_start(
                        bt[:, j:j + 1],
                        bd.ap()[j:j + 1, :].rearrange("a p -> p a"))
                b_sb[name] = bt

            coef_sb = cpool.tile([128, 3], F32, name="coef_sb")
            nc.sync.dma_start(coef_sb[:], coef.ap())

            # cz: col 0 = SEG/cb (z accumulation factor), col 1 = 1/cd
            # (softmax denominator scale); constant over partitions.
            cz = cpool.tile([128, 2], BF16, name="cz")
            nc.vector.tensor_copy(cz[:, 0:1], coef_sb[:, 2:3])
            nc.vector.tensor_copy(cz[:, 1:2], coef_sb[:, 1:2])

            # out_acc initializer: the folded v-bias row replicated into all
            # four l-blocks (see bias_out above).
            oinit = cpool.tile([128, 4 * DV], F32, name="oinit")
            for lb in range(4):
                nc.sync.dma_start(oinit[:, lb * DV:(lb + 1) * DV],
                                  bias_out.ap())

            # memory state per head, bf16: cols 0:128 = mem, col 128 = z
            # (z pre-scaled by 1/cb via the SEG/cb constant column).
            mem_bf = cpool.tile([128, HPC * 129], BF16, name="mem_bf")

            # two persistent v_ext buffers (alternate per segment); constant
            # cols written once here. Blocks of 160 cols: 0:128 = v,
            # col 128 = SEG/cb, col 129 = 1/cd, 130:160 unused.
            v_ext_ab = []
            for i in range(2):
                ve = cpool.tile([128, HPC * 4 * 160], BF16, name=f"vext{i}")
                for blk in range(HPC * 4):
                    nc.gpsimd.tensor_copy(
                        ve[:, blk * 160 + 128:blk * 160 + 130], cz[:])
                v_ext_ab.append(ve)

            # ---------- software-pipelined segment loop helpers ----------
            def alloc_seg(rep, t):
                u = rep * NSEG + t  # unique id for tile names
                st = {"t": t}
                st["xt"] = wpool.tile([128, NDT * 512], F32R,
                                      name=f"xtr{u}", tag="xtr")
                if "no_xload" not in ABL or u < 2:
                    nc.sync.dma_start(
                        st["xt"][:].rearrange("p (dt s) -> p dt s", s=512),
                        xT.ap()[:, t * 512:(t + 1) * 512]
                          .rearrange("(dt p) s -> p dt s", p=128))
                st["qT"] = wpool.tile([128, HPC * 512], BF16,
                                      name=f"qT{u}", tag="qT")
                st["kT"] = wpool.tile([128, HPC * 512], BF16,
                                      name=f"kT{u}", tag="kT")
                st["sqT"] = wpool.tile([128, HPC * 512], BF16,
                                       name=f"sqT{u}", tag="sqT")
                st["skT"] = wpool.tile([128, HPC * 512], BF16,
                                       name=f"skT{u}", tag="skT")
                st["skn"] = wpool.tile([128, HPC * 512], BF16,
                                       name=f"skn{u}", tag="skn")
                st["v_ext"] = v_ext_ab[t % 2]
                st["oacc"] = wpool.tile([128, 4 * 128], F32,
                                        name=f"oac{u}", tag="oacc")
                nc.gpsimd.tensor_copy(st["oacc"][:], oinit[:])
                st["u"] = u
                return st

            def emit_kq_group(st, name, hb):
                if "no_kq" in ABL:
                    return
                u = st["u"]
                wr = w_r[name]
                pp = proj_ps.tile([128, 512], F32,
                                  name=f"pp{u}_{name}{hb}", tag="proj")
                for dt in range(NDT):
                    nc.tensor.matmul(
                        pp[:],
                        wr[:, dt * 512 + hb * 128:dt * 512 + (hb + 1) * 128],
                        st["xt"][:, dt * 512:(dt + 1) * 512],
                        start=(dt == 0), stop=(dt == NDT - 1))
                bias = b_sb[name][:, hb:hb + 1]
                sl = slice(hb * 512, (hb + 1) * 512)
                raw = st["qT"] if name == "q" else st["kT"]
                s_out = st["sqT"] if name == "q" else st["skT"]
                # raw projection with bias (bf16, scores operand); the only
                # psum reader. k-raws on ACT, q-raws on DVE to balance.
                if "no_raw" not in ABL:
                    if name == "k":
                        nc.scalar.activation(raw[:, sl], pp[:], AF.Identity,
                                             bias=bias)
                    else:
                        nc.vector.tensor_scalar(raw[:, sl], pp[:], bias, None,
                                                op0=ALU.add)
                # elu(x)+1 = relu(x) + min(exp(x), 1), computed from the
                # bf16 raw (bias applied) so DVE runs in 2x bf16 mode.
                if "no_feat" not in ABL:
                    e_t = spool.tile([128, 512], BF16,
                                     name=f"e{u}_{name}{hb}", tag="e", bufs=3)
                    nc.scalar.activation(e_t[:], raw[:, sl], AF.Exp)
                    r_t = spool.tile([128, 512], BF16,
                                     name=f"r{u}_{name}{hb}", tag="r", bufs=3)
                    nc.vector.tensor_scalar_max(r_t[:], raw[:, sl], 0.0)
                    # s_out = min(e, 1) + r in one fused op
                    nc.vector.scalar_tensor_tensor(
                        s_out[:, sl], e_t[:], 1.0, r_t[:],
                        op0=ALU.min, op1=ALU.add)

            def emit_sk_transpose(st):
                # one partition-tiled dma transpose for all 4 heads:
                # skT [128 dk, (h l)] -> sk_nat [128 l, (h lt), 128 dk]
                if "no_tp" in ABL:
                    return
                nc.sync.dma_start_transpose(
                    st["skn"][:].rearrange("p (b c) -> p b c", c=128),
                    st["skT"][:])

            def emit_v_group(st, lt):
                if "no_v" in ABL:
                    return
                u = st["u"]
                wr = w_r["v"]
                pp = proj_ps.tile([128, 512], F32,
                                  name=f"pp{u}_v{lt}", tag="proj")
                for dt in range(NDT):
                    nc.tensor.matmul(
                        pp[:],
                        st["xt"][:, dt * 512 + lt * 128:dt * 512 + (lt + 1) * 128],
                        wr[:, dt * 512:(dt + 1) * 512],
                        start=(dt == 0), stop=(dt == NDT - 1))
                # scatter the 4 head blocks into v_ext (block id lt*4+h)
                if "no_vscat" not in ABL:
                    nc.scalar.activation(
                        st["v_ext"][:, lt * 640:(lt + 1) * 640]
                        .rearrange("p (h c) -> p h c", c=160)[:, :, 0:128],
                        pp[:].rearrange("p (h c) -> p h c", c=128),
                        AF.Copy)

            def emit_attn_scores(st, h):
                # scores^T -> exp, emitted one pipeline chunk ahead of
                # emit_attn_rest(h) so the ACT exps never gate PE's adot
                u = st["u"]
                exp_sb = wpool.tile([128, 4 * 512], BF16,
                                    name=f"ex{u}_{h}", tag="exp")
                st[f"exp{h}"] = exp_sb
                if "no_scores" in ABL:
                    return
                kT, qT = st["kT"], st["qT"]
                hsl = slice(h * 512, (h + 1) * 512)
                for mb in range(4):
                    sps = score_ps.tile([128, 512], F32,
                                        name=f"sc{u}_{h}{mb}", tag="score")
                    nc.tensor.matmul(
                        sps[:],
                        kT[:, h * 512 + mb * 128:h * 512 + (mb + 1) * 128],
                        qT[:, hsl], start=True, stop=True)
                    if "no_exp" not in ABL:
                        nc.scalar.activation(
                            exp_sb[:, mb * 512:(mb + 1) * 512],
                            sps[:], AF.Exp, scale=SCALE)

            def emit_attn_rest(st, h):
                u = st["u"]
                sqT, sk_nat = st["sqT"], st["skn"]
                v_ext, out_acc = st["v_ext"], st["oacc"]
                exp_sb = st[f"exp{h}"]
                # memory update; the bf16 accumulate hides behind the
                # first adot group below
                msl = slice(h * 129, (h + 1) * 129)
                if "no_update" not in ABL:
                    up = upd_ps.tile([128, 129], F32, name=f"up{u}_{h}",
                                     tag="sml")
                    for lt in range(4):
                        blk = slice((h * 4 + lt) * 128, (h * 4 + lt + 1) * 128)
                        vbase = (lt * 4 + h) * 160
                        nc.tensor.matmul(
                            up[:], sk_nat[:, blk],
                            v_ext[:, vbase:vbase + 129],
                            start=(lt == 0), stop=(lt == 3))
                    nc.vector.tensor_add(mem_bf[:, msl], mem_bf[:, msl],
                                         up[:])
                for lb in range(4):
                    lsl = slice(h * 512 + lb * 128, h * 512 + (lb + 1) * 128)
                    adp = None
                    if "no_adot" not in ABL:
                        adp = adot_ps.tile([128, 130], F32,
                                           name=f"ad{u}_{h}{lb}", tag="sml")
                        for mt in range(4):
                            esl = slice(mt * 512 + lb * 128,
                                        mt * 512 + (lb + 1) * 128)
                            vbase = (mt * 4 + h) * 160
                            nc.tensor.matmul(
                                adp[:], exp_sb[:, esl],
                                v_ext[:, vbase:vbase + 130],
                                start=(mt == 0), stop=(mt == 3))
                    rps = None
                    if "no_ret" not in ABL:
                        rps = ret_ps.tile([128, 129], F32,
                                          name=f"rt{u}_{h}{lb}", tag="sml")
                        nc.tensor.matmul(rps[:], sqT[:, lsl], mem_bf[:, msl],
                                         start=True, stop=True)
                    if "no_epi" in ABL:
                        continue
                    # epilogue: out += cb*numer/denz + numer2/dend
                    # (reference's +1e-5 is negligible: denz >= ~1e5)
                    osl = out_acc[:, lb * 128:(lb + 1) * 128]
                    if rps is not None:
                        rz = spool.tile([128, 1], F32, name=f"rz{u}_{h}{lb}",
                                        tag="rz", bufs=8)
                        nc.vector.reciprocal(rz[:], rps[:, 128:129])
                        nc.vector.scalar_tensor_tensor(
                            osl, rps[:, 0:128], rz[:], osl,
                            op0=ALU.mult, op1=ALU.add)
                    if adp is not None:
                        rd = spool.tile([128, 1], F32, name=f"rd{u}_{h}{lb}",
                                        tag="rd", bufs=8)
                        nc.vector.reciprocal(rd[:], adp[:, 129:130])
                        nc.vector.scalar_tensor_tensor(
                            osl, adp[:, 0:128], rd[:], osl,
                            op0=ALU.mult, op1=ALU.add)

            def emit_out_store(st):
                if "no_store" in ABL:
                    return
                t = st["t"]
                nc.gpsimd.dma_start(
                    out.ap()[t * 512:(t + 1) * 512, :]
                       .rearrange("(lt p) v -> p lt v", p=128),
                    st["oacc"][:].rearrange("p (lt v) -> p lt v", v=128))

            # ---------- pipelined emission: proj(t) interleaved with
            # attention(t-1); scores+exp for head h+1 are emitted a full
            # chunk before adot/ret of head h so ACT exp latency never
            # gates PE ----------
            for rep in range(REPS):
                nc.vector.memset(mem_bf[:], 0.0)
                prev = None
                for t in range(NSEG):
                    st = alloc_seg(rep, t)
                    if prev is not None:
                        emit_attn_scores(prev, 1)
                    emit_kq_group(st, "k", 0)
                    emit_kq_group(st, "k", 1)
                    if prev is not None:
                        emit_attn_rest(prev, 0)
                        emit_attn_scores(prev, 2)
                    emit_kq_group(st, "k", 2)
                    emit_kq_group(st, "k", 3)
                    if prev is not None:
                        emit_attn_rest(prev, 1)
                        emit_attn_scores(prev, 3)
                    emit_sk_transpose(st)
                    emit_kq_group(st, "q", 0)
                    emit_kq_group(st, "q", 1)
                    if prev is not None:
                        emit_attn_rest(prev, 2)
                    emit_kq_group(st, "q", 2)
                    emit_kq_group(st, "q", 3)
                    if prev is not None:
                        emit_attn_rest(prev, 3)
                    emit_v_group(st, 0)
                    emit_v_group(st, 1)
                    if prev is not None:
                        emit_out_store(prev)
                    emit_v_group(st, 2)
                    emit_v_group(st, 3)
                    # head-0 scores of the current segment close the loop
                    emit_attn_scores(st, 0)
                    prev = st
                # drain: attention of the last segment
                emit_attn_rest(prev, 0)
                for h in range(1, HPC):
                    emit_attn_scores(prev, h)
                    emit_attn_rest(prev, h)
                emit_out_store(prev)

    nc.compile()
    return nc


def _get_compiled():
    if "nc" not in _CACHE:
        _CACHE["nc"] = _build()
    return _CACHE["nc"]


def make_in_maps(x, wq, bq, wk, bk, wv, bv, beta):
    bsig = float(1.0 / (1.0 + np.exp(-np.float64(beta[0]))))
    cb = bsig / H_TOT
    cd = (1.0 - bsig) / H_TOT
    coef = np.empty((128, 3), np.float32)
    coef[:, 0] = cb
    coef[:, 1] = (1.0 / cd) if cd != 0.0 else np.inf
    coef[:, 2] = SEG / cb

    xT_by_b = [np.ascontiguousarray(x[b].T).astype(np.float32, copy=False)
               for b in range(B)]
    in_maps = []
    for c in range(NCORES):
        b, hg = c // 2, c % 2
        sl = slice(hg * HPC * DK, (hg + 1) * HPC * DK)
        bv_core = np.ascontiguousarray(bv[sl]).reshape(HPC, DV)
        brow = (cb / SEG + cd) * bv_core.sum(axis=0)
        bias_out = np.broadcast_to(brow.astype(np.float32),
                                   (128, DV)).copy()
        in_maps.append({
            "xT": xT_by_b[b],
            "wq": np.ascontiguousarray(wq[:, sl]),
            "wk": np.ascontiguousarray(wk[:, sl]),
            "wv": np.ascontiguousarray(wv[:, sl]),
            "bq": np.ascontiguousarray(bq[sl]).reshape(HPC, DK),
            "bk": np.ascontiguousarray(bk[sl]).reshape(HPC, DK),
            "bias_out": bias_out,
            "coef": coef,
        })
    return in_maps


def kernel(x, wq, bq, wk, bk, wv, bv, beta):
    nc = _get_compiled()
    in_maps = make_in_maps(x, wq, bq, wk, bk, wv, bv, beta)
    res = run_bass_kernel_spmd(nc, in_maps, core_ids=list(range(NCORES)))
    out = np.empty((B, S, DV), np.float32)
    for b in range(B):
        out[b] = res.results[2 * b]["out"] + res.results[2 * b + 1]["out"]
    return out


if __name__ == "__main__":
    rng = np.random.default_rng(0)
    x = rng.normal(size=(B, S, D)).astype(np.float32)
    sc = 1.0 / np.sqrt(D)
    wq_ = (rng.normal(size=(D, 1024)) * sc).astype(np.float32)
    wk_ = (rng.normal(size=(D, 1024)) * sc).astype(np.float32)
    wv_ = (rng.normal(size=(D, 1024)) * sc).astype(np.float32)
    bq_ = (rng.normal(size=(1024,)) * 0.01).astype(np.float32)
    bk_ = (rng.normal(size=(1024,)) * 0.01).astype(np.float32)
    bv_ = (rng.normal(size=(1024,)) * 0.01).astype(np.float32)
    beta_ = np.zeros((1,), np.float32)
    o = kernel(x, wq_, bq_, wk_, bk_, wv_, bv_, beta_)
    print("out", o.shape, o.dtype, float(np.abs(o).max()))

